# revision 6
# baseline (speedup 1.0000x reference)
"""Trainium2 Bass kernel for nn_AttentionSimilarity (v2, fused).

Reference computation (per batch element, B=8 data-parallel over 8 cores):
    q_in = pairwise-mean(x)            # [M, D], M = N/2
    q    = q_in @ Wq.T + bq            # [M, D]
    k    = x @ Wk.T + bk               # [N, D]
    v    = x @ Wv.T + bv               # [N, D]
    attn = softmax(q @ k.T / sqrt(D))  # [M, N]
    o    = attn @ v                    # [M, D]
    return (q, o, o)

v2 strategy (per core), all attention matmul operands bf16:
  Reassociation: o = (P @ x) @ Wv.T + bv  (v projection eliminated;
  row-normalization commutes since (sum_n P)/r == 1).

  Setup : transpose Wk, Wq, Wv to [d-part, e] bf16 chunks; bias layouts.
  A     : single pass over x (strips of 256 rows): convert x to bf16
          (resident x_sb), PE-transpose to xT strips, kT = WkT.T @ xT
          into resident kT_sb [e, n]; pool adjacent xT columns -> xq;
          qT = (0.5*WqT).T @ xq -> spill to qT_dram (bf16).
  BC    : fused attention per m-super (256 columns of qT):
          - load qT chunk; PE-transpose it -> q natural output (DMA out)
          - per n-chunk i (32): S^T = kT_i.T @ qTc (PSUM, 8 e-chunk MMs),
            P^T = exp(S * scale) (ACT -> bf16)
            PX[j] += P^T_j.T @ x_i (PSUM accumulate over all i)
            r[j]  += P^T_j.T @ ones2 (rider MM, same stationary)
          - Obar[j] = PX[j] * (1/r[j]) (DVE, -> bf16)
          - PE-transpose Obar -> ObT [d, m]; o = ObT.T @ WvT + bv -> DMA.

Softmax max-subtraction is skipped: logits are ~N(0, 0.7^2) so exp is
safe in fp32 and the result is mathematically identical.
"""

import sys

if "/opt/trn_rl_repo" not in sys.path:
    sys.path.insert(0, "/opt/trn_rl_repo")

from contextlib import ExitStack

import numpy as np

import concourse.bass as bass
import concourse.mybir as mybir
import concourse.tile as tile
from concourse import bacc
from concourse.masks import make_identity

F32 = mybir.dt.float32
BF = mybir.dt.bfloat16
AF = mybir.ActivationFunctionType
P = 128

# Full-problem constants
FULL_B, FULL_N, FULL_D = 8, 4096, 1024


def build_program(
    N=FULL_N,
    D=FULL_D,
    repeats=1,
    tune=None,
    **tune_kw,
):
    """Build the per-core SPMD Bass program. Every core runs the same
    program on its own batch element; no collectives."""
    M = N // 2
    DC = D // P        # feature chunks of 128 (8)
    NC = N // P        # key chunks of 128 (32)
    MC = M // P        # query chunks of 128 (16)
    SCALE = float(D) ** -0.5

    T = dict(
        nstrip=256,     # x rows per phase-A strip
        msup=256,       # m columns per BC super-block
        xn_bufs=2, xt_bufs=2, xq_bufs=1, qts_bufs=1,
        qtc_bufs=2, pt_bufs=3, qev_bufs=2, obar_bufs=2, obt_bufs=2,
        oout_bufs=2,
        work_ps=3, px_ps=2, kq_ps=2, t_ps=3,
    )
    if tune:
        T.update(tune)
    if tune_kw:
        T.update(tune_kw)

    nstrip = T["nstrip"]
    SJ = nstrip // P   # 128-row blocks per strip (2)
    SN = N // nstrip   # strips (16)
    MSUP = T["msup"]
    GM = M // MSUP     # super-blocks (8)
    MB = MSUP // P     # m sub-blocks per super (2)
    QB = MSUP // (nstrip // 2)  # strips per q-batch (2)

    nc = bacc.Bacc("TRN2", target_bir_lowering=False, debug=False)

    x_d = nc.dram_tensor("x", [N, D], F32, kind="ExternalInput").ap()
    wq_d = nc.dram_tensor("Wq", [D, D], F32, kind="ExternalInput").ap()
    bq_d = nc.dram_tensor("bq", [D], F32, kind="ExternalInput").ap()
    wk_d = nc.dram_tensor("Wk", [D, D], F32, kind="ExternalInput").ap()
    bk_d = nc.dram_tensor("bk", [D], F32, kind="ExternalInput").ap()
    wv_d = nc.dram_tensor("Wv", [D, D], F32, kind="ExternalInput").ap()
    bv_d = nc.dram_tensor("bv", [D], F32, kind="ExternalInput").ap()
    q_d = nc.dram_tensor("q", [M, D], F32, kind="ExternalOutput").ap()
    o_d = nc.dram_tensor("o", [M, D], F32, kind="ExternalOutput").ap()

    def mm(ps, lhsT, rhs, start, stop):
        nc.tensor.matmul(ps, lhsT, rhs, start=start, stop=stop)

    with tile.TileContext(nc) as tc, ExitStack() as ctx:
        const = ctx.enter_context(tc.tile_pool(name="const", bufs=1))
        dram = ctx.enter_context(tc.tile_pool(name="dram", bufs=1, space="DRAM"))

        qT_dram = dram.tile([P, DC, M], BF)

        identity = const.tile([P, P], F32)
        make_identity(nc, identity)
        identity_bf = const.tile([P, P], BF)
        nc.vector.tensor_copy(identity_bf, identity)
        ones2_f32 = const.tile([P, 2], F32)
        nc.vector.memset(ones2_f32, 1.0)
        ones2_bf = const.tile([P, 2], BF)
        nc.vector.tensor_copy(ones2_bf, ones2_f32)
        ones_row = const.tile([1, P], F32)
        nc.vector.memset(ones_row, 1.0)

        # ---- residents ----
        res_pool = ctx.enter_context(tc.tile_pool(name="res", bufs=1))
        kT_sb = res_pool.tile([P, DC, N], BF)    # k^T: [e-part, ec, n]
        x_sb = res_pool.tile([P, NC, D], BF)     # x:   [n-part, nchunk, d]
        wvT = res_pool.tile([P, DC, D], BF)      # Wv^T: [d-part, dc, e]

        # ---- biases ----
        with ExitStack() as bias_ctx:
            bpsum = bias_ctx.enter_context(
                tc.tile_pool(name="bpsum", bufs=2, space="PSUM"))
            brow_pool = bias_ctx.enter_context(tc.tile_pool(name="brow", bufs=2))

            # per-partition layouts [P, DC]: b_sb[p, c] = b[c*128 + p]
            def bias_cols(b_d, name):
                brow = brow_pool.tile([DC, P], F32, tag="brow")
                nc.sync.dma_start(brow, b_d.rearrange("(c p) -> c p", p=P))
                ps = bpsum.tile([P, DC], F32, tag="bps")
                nc.tensor.transpose(ps, brow, identity[:DC, :DC])
                b_sb = const.tile([P, DC], F32, tag=f"bias_{name}")
                nc.vector.tensor_copy(b_sb, ps)
                return b_sb

            bk_sb = bias_cols(bk_d, "bk")
            bq_sb = bias_cols(bq_d, "bq")

            # broadcast layout [P, D] (same bias row on every partition),
            # built with a K=1 ones-matmul: out = ones[1,P].T @ b[1,D]
            brow = brow_pool.tile([1, D], F32, tag="brow_flat")
            nc.sync.dma_start(brow, bv_d[None, :])
            bv_bc = const.tile([P, D], F32)
            for g in range(2):
                ps = bpsum.tile([P, D // 2], F32, tag="bbc_ps")
                nc.tensor.matmul(
                    ps, ones_row, brow[:, g * (D // 2):(g + 1) * (D // 2)],
                    start=True, stop=True)
                nc.vector.tensor_copy(bv_bc[:, g * (D // 2):(g + 1) * (D // 2)], ps)

        # ---- transposed weights (bf16) ----
        # wT[p, dc, e] = W[e, dc*128+p]  (contraction dim d on partitions)
        def load_wT(w_d, wT, tpsum, wnat_pool, scale=None):
            for ec in range(DC):
                wn = wnat_pool.tile([P, D], F32, tag="wnat")
                nc.sync.dma_start(wn, w_d[ec * P:(ec + 1) * P, :])
                wb = wnat_pool.tile([P, D], BF, tag="wbf")
                if scale is None:
                    nc.vector.tensor_copy(wb, wn)
                else:
                    nc.vector.tensor_scalar_mul(wb, wn, scale)
                for dc in range(DC):
                    ps = tpsum.tile([P, P], BF, tag="tps")
                    nc.tensor.transpose(ps, wb[:, dc * P:(dc + 1) * P], identity_bf)
                    nc.vector.tensor_copy(wT[:, dc, ec * P:(ec + 1) * P], ps)

        for _rep in range(repeats):
            tc.no_sync_barrier()
            # =================== Phase A ===================
            with ExitStack() as actx:
                wkq_pool = actx.enter_context(tc.tile_pool(name="wkq", bufs=1))
                tpsum = actx.enter_context(
                    tc.tile_pool(name="tpsum", bufs=T["t_ps"], space="PSUM"))
                wkT = wkq_pool.tile([P, DC, D], BF, tag="wkT")
                wqT = wkq_pool.tile([P, DC, D], BF, tag="wqT")
                with ExitStack() as wctx:
                    wnat_pool = wctx.enter_context(tc.tile_pool(name="wnat", bufs=2))
                    load_wT(wk_d, wkT, tpsum, wnat_pool)
                    # 0.5 from pair-mean pooling folded into Wq
                    load_wT(wq_d, wqT, tpsum, wnat_pool, scale=0.5)
                    load_wT(wv_d, wvT, tpsum, wnat_pool)

                xn_pool = actx.enter_context(
                    tc.tile_pool(name="xn", bufs=T["xn_bufs"]))
                xT_pool = actx.enter_context(
                    tc.tile_pool(name="xT", bufs=T["xt_bufs"]))
                xq_pool = actx.enter_context(
                    tc.tile_pool(name="xq", bufs=T["xq_bufs"]))
                qts_pool = actx.enter_context(
                    tc.tile_pool(name="qts", bufs=T["qts_bufs"]))
                kpsum = actx.enter_context(
                    tc.tile_pool(name="kpsum", bufs=T["kq_ps"], space="PSUM"))
                qpsum = actx.enter_context(
                    tc.tile_pool(name="qpsum", bufs=T["kq_ps"], space="PSUM"))

                mw = nstrip // 2  # pooled columns per strip
                xq = None
                for s in range(SN):
                    # load + convert strip (SJ 128-row blocks)
                    xT = xT_pool.tile([P, DC, nstrip], BF, tag="xT")
                    for j in range(SJ):
                        i = s * SJ + j  # n-chunk index
                        xn = xn_pool.tile([P, D], F32, tag="xn")
                        nc.sync.dma_start(xn, x_d[i * P:(i + 1) * P, :])
                        nc.vector.tensor_copy(x_sb[:, i, :], xn)
                        for dc in range(DC):
                            ps = tpsum.tile([P, P], BF, tag="tps")
                            nc.tensor.transpose(
                                ps, x_sb[:, i, dc * P:(dc + 1) * P], identity_bf)
                            nc.vector.tensor_copy(
                                xT[:, dc, j * P:(j + 1) * P], ps)
                    # k^T strip: [e, n] chunks; bias added on eviction
                    for ec in range(DC):
                        ps = kpsum.tile([P, nstrip], F32, tag="kps")
                        for dc in range(DC):
                            mm(ps, wkT[:, dc, ec * P:(ec + 1) * P], xT[:, dc, :],
                               start=(dc == 0), stop=(dc == DC - 1))
                        nc.scalar.activation(
                            kT_sb[:, ec, s * nstrip:(s + 1) * nstrip], ps,
                            AF.Identity, bias=bk_sb[:, ec:ec + 1])
                    # adjacent-pair pooling (0.5 folded into Wq already)
                    if s % QB == 0:
                        xq = xq_pool.tile([P, DC, QB * mw], BF, tag="xq")
                    for dc in range(DC):
                        v2 = xT[:, dc, :].rearrange("p (m two) -> p m two", two=2)
                        nc.vector.tensor_add(
                            xq[:, dc, (s % QB) * mw:(s % QB + 1) * mw],
                            v2[:, :, 0], v2[:, :, 1])
                    # q^T batch (every QB strips): [e, m] -> DRAM
                    if s % QB == QB - 1:
                        moff = (s // QB) * QB * mw
                        qts = qts_pool.tile([P, DC, QB * mw], BF, tag="qts")
                        for ec in range(DC):
                            ps = qpsum.tile([P, QB * mw], F32, tag="qps")
                            for dc in range(DC):
                                mm(ps, wqT[:, dc, ec * P:(ec + 1) * P], xq[:, dc, :],
                                   start=(dc == 0), stop=(dc == DC - 1))
                            nc.scalar.activation(
                                qts[:, ec, :], ps, AF.Identity,
                                bias=bq_sb[:, ec:ec + 1])
                        nc.sync.dma_start(
                            qT_dram[:, :, moff:moff + QB * mw], qts)

            # =================== Phase BC (fused attention) ===================
            with ExitStack() as bctx:
                qtc_pool = bctx.enter_context(
                    tc.tile_pool(name="qtc", bufs=T["qtc_bufs"]))
                pt_pool = bctx.enter_context(
                    tc.tile_pool(name="pt", bufs=T["pt_bufs"]))
                qev_pool = bctx.enter_context(
                    tc.tile_pool(name="qev", bufs=T["qev_bufs"]))
                obar_pool = bctx.enter_context(
                    tc.tile_pool(name="obar", bufs=T["obar_bufs"]))
                obt_pool = bctx.enter_context(
                    tc.tile_pool(name="obt", bufs=T["obt_bufs"]))
                oout_pool = bctx.enter_context(
                    tc.tile_pool(name="oout", bufs=T["oout_bufs"]))
                rinv_pool = bctx.enter_context(tc.tile_pool(name="rinv", bufs=2))
                work_ps = bctx.enter_context(
                    tc.tile_pool(name="work_ps", bufs=T["work_ps"], space="PSUM"))
                px_ps = bctx.enter_context(
                    tc.tile_pool(name="px_ps", bufs=T["px_ps"], space="PSUM"))
                r_ps_pool = bctx.enter_context(
                    tc.tile_pool(name="r_ps", bufs=1, space="PSUM"))

                for g in range(GM):
                    m0 = g * MSUP
                    qtc = qtc_pool.tile([P, DC, MSUP], BF, tag="qtc")
                    nc.sync.dma_start(qtc, qT_dram[:, :, m0:m0 + MSUP])

                    # q natural output: PE-transpose the qT chunk.
                    # Transpose outputs borrow work_ps tiles via a bf16
                    # bitcast view so the pool stays single-tag (1 bank/slot).
                    for jj in range(MB):
                        qev = qev_pool.tile([P, DC, P], F32, tag="qev")
                        for ec in range(DC):
                            w = work_ps.tile([P, MSUP], F32, tag="sps", name="qtp")
                            ps = w.bitcast(BF)[:, 0:P]
                            nc.tensor.transpose(
                                ps, qtc[:, ec, jj * P:(jj + 1) * P], identity_bf)
                            nc.vector.tensor_copy(qev[:, ec, :], ps)
                        nc.sync.dma_start(
                            q_d[m0 + jj * P:m0 + (jj + 1) * P, :],
                            qev.rearrange("p c w -> p (c w)"))

                    # attention: S^T -> exp -> PX/r accumulation
                    pxs = [px_ps.tile([P, D], F32, tag="px", name=f"px{_j}")
                           for _j in range(MB)]
                    r_ps = r_ps_pool.tile([P, 2 * MB], F32)
                    for i in range(NC):
                        sps = work_ps.tile([P, MSUP], F32, tag="sps")
                        for ec in range(DC):
                            mm(sps, kT_sb[:, ec, i * P:(i + 1) * P], qtc[:, ec, :],
                               start=(ec == 0), stop=(ec == DC - 1))
                        pt = pt_pool.tile([P, MSUP], BF, tag="pt")
                        nc.scalar.activation(pt, sps, AF.Exp, scale=SCALE)
                        for j in range(MB):
                            ptj = pt[:, j * P:(j + 1) * P]
                            for h in range(2):
                                mm(pxs[j][:, h * (D // 2):(h + 1) * (D // 2)],
                                   ptj, x_sb[:, i, h * (D // 2):(h + 1) * (D // 2)],
                                   start=(i == 0), stop=(i == NC - 1))
                            mm(r_ps[:, 2 * j:2 * j + 2], ptj, ones2_bf,
                               start=(i == 0 and j == 0),
                               stop=(i == NC - 1 and j == MB - 1))

                    # normalize, transpose, project with WvT, add bv, out
                    for j in range(MB):
                        rinv = rinv_pool.tile([P, 1], F32, tag="rinv")
                        nc.vector.reciprocal(rinv, r_ps[:, 2 * j:2 * j + 1])
                        obar = obar_pool.tile([P, D], BF, tag="obar")
                        nc.vector.tensor_scalar_mul(obar, pxs[j], rinv[:, 0:1])
                        obt = obt_pool.tile([P, DC, P], BF, tag="obt")
                        for dc in range(DC):
                            w = work_ps.tile([P, MSUP], F32, tag="sps", name="otp")
                            ps = w.bitcast(BF)[:, 0:P]
                            nc.tensor.transpose(
                                ps, obar[:, dc * P:(dc + 1) * P], identity_bf)
                            nc.vector.tensor_copy(obt[:, dc, :], ps)
                        ops = px_ps.tile([P, D], F32, tag="px")
                        for h in range(2):
                            for dc in range(DC):
                                mm(ops[:, h * (D // 2):(h + 1) * (D // 2)],
                                   obt[:, dc, :],
                                   wvT[:, dc, h * (D // 2):(h + 1) * (D // 2)],
                                   start=(dc == 0), stop=(dc == DC - 1))
                        oout = oout_pool.tile([P, D], F32, tag="oout")
                        nc.vector.tensor_add(oout, ops, bv_bc)
                        nc.sync.dma_start(
                            o_d[m0 + j * P:m0 + (j + 1) * P, :], oout)

    nc.compile()
    return nc


_program_cache = {}


def _get_program(key=("full",), **kwargs):
    if key not in _program_cache:
        _program_cache[key] = build_program(**kwargs)
    return _program_cache[key]


def run(inputs, trace=False, **build_kwargs):
    """inputs: dict with full-shape arrays. Returns (results, BassKernelResults)."""
    from concourse.bass_utils import run_bass_kernel_spmd

    x = np.ascontiguousarray(np.asarray(inputs["x"], dtype=np.float32))
    B = x.shape[0]
    weights = {
        k: np.ascontiguousarray(np.asarray(inputs[k], dtype=np.float32))
        for k in ("Wq", "bq", "Wk", "bk", "Wv", "bv")
    }
    key = ("full",) if not build_kwargs else tuple(sorted(build_kwargs.items()))
    nc = _get_program(key=key, **build_kwargs)
    in_maps = [dict(x=x[i], **weights) for i in range(B)]
    res = run_bass_kernel_spmd(nc, in_maps, list(range(B)), trace=trace)
    q = np.stack([res.results[i]["q"] for i in range(B)])
    o = np.stack([res.results[i]["o"] for i in range(B)])
    return (q, o), res


def kernel(x, Wq, bq, Wk, bk, Wv, bv):
    (q, o), _ = run(dict(x=x, Wq=Wq, bq=bq, Wk=Wk, bk=bk, Wv=Wv, bv=bv))
    return (q, o, o)
